# revision 11
# baseline (speedup 1.0000x reference)
"""Trainium2 Bass kernel for nn_Conv4Pim_group_arr_v3 (PIM-style grouped quantized conv).

Computation (see reference):
  - x [16,256,56,56] f32, weight [256,256,3,3], per-group (G=4, 64 ic each) LSQ
    quantization: weights to integer levels {0..3} (pos/neg split), partial-sum conv
    outputs rounded to int levels in [-128,127] and rescaled, accumulated over groups.

Strategy: data-parallel over batch (2 images per core, 8 cores, no collectives).
fp8 DoubleRow matmuls: x is split exactly into two e4m3 lanes (x ~ a/4 + b/128 with
a,b integers, both lanes exact in fp8; |x|>4.125 tail uses an even grid, still exact),
weights are integer levels {0..3} (exact in fp8). Each DoubleRow pass contracts both
lanes (2 K-tiles of 128) per column, so full x precision costs no extra passes.

x is stored as a padded [58 rows, 58 cols] grid per (img, group), two shifted copies
(T1 = [A | A+1col] and T2 = [A | A+1row] stacked in the K partition dim) so conv taps
pair up. All matmul reads are rectangular [8 rows, 56 cols] slices -> 448 columns per
pass (no pad-column waste).

Per core, per (img, group, och-tile-of-512, sptile-of-8-rows):
  - 5 fp8 DoubleRow matmuls accumulate the 3x3 conv into one PSUM tile [128, 448].
  - ACT: Copy(psum * (w_scale/ps_scale)) with int8 output = round-half-even +
    saturate to [-128,127] in one op == the LSQ psum quantizer.
  - DVE scalar_tensor_tensor: acc_fp16 += q_int8 * (+-ps_scale).
Output fp16 -> host f32.
"""

import numpy as np
import ml_dtypes

import concourse.mybir as mybir
import concourse.tile as tile
from concourse import bacc
from concourse.bass_utils import run_bass_kernel_spmd

F32 = mybir.dt.float32
F16 = mybir.dt.float16
F8 = mybir.dt.float8e4
I8 = mybir.dt.int8
NP_F8 = ml_dtypes.float8_e4m3

B, IC, H, W = 16, 256, 56, 56
OC = 256
G = 4
CG = 64  # ic per group
K = 3
QP_W = 3  # 2**2 - 1
N_CORES = 8
BPC = B // N_CORES  # images per core

PW = W + 2  # 58 padded width
PH = H + 2  # 58 padded height
FLAT = PW * PH  # 3364
SP = 7  # spatial tiles of 8 output rows
ROWS = 8
NCOL = ROWS * W  # 448 columns per psum tile
OC4 = 4  # och tiles of 128 over 512 (pos|neg x 256)
NSL = G * OC4 * 5  # weight slices, each [128 K, 2 lanes, 128 M]

_nc_cache = {}


def _build_nc():
    nc = bacc.Bacc(
        "TRN2",
        target_bir_lowering=False,
        debug=False,
        enable_asserts=True,
        num_devices=N_CORES,
    )

    xt1_d = nc.dram_tensor("xt1", [BPC, G, 128, 2, PH, PW], F8, kind="ExternalInput").ap()
    xt2_d = nc.dram_tensor("xt2", [BPC, G, 128, 2, PH, PW], F8, kind="ExternalInput").ap()
    wts_d = nc.dram_tensor("wts", [128, NSL, 2, 128], F8, kind="ExternalInput").ap()
    scl_d = nc.dram_tensor("scl", [128, 2 * G * OC4], F32, kind="ExternalInput").ap()
    out_d = nc.dram_tensor("out", [BPC, 2, 128, SP, NCOL], F16, kind="ExternalOutput").ap()

    DR = mybir.MatmulPerfMode.DoubleRow

    with tile.TileContext(nc) as tc:
        with (
            tc.tile_pool(name="xp", bufs=1) as xp,
            tc.tile_pool(name="wp", bufs=1) as wp,
            tc.tile_pool(name="accp", bufs=2) as accp,
            tc.tile_pool(name="qp", bufs=8) as qp,
            tc.tile_pool(name="psum", bufs=8, space="PSUM") as pp,
        ):
            wts = wp.tile([128, NSL, 2, 128], F8, tag="wts")
            scl = wp.tile([128, 2 * G * OC4], F32, tag="scl")
            # Startup-critical DMA schedule over two queues (sync = HWDGE, gpsimd =
            # SWDGE). The (img0, g0) block runs oc4-outer: each oc4 sweep (7 sp tiles,
            # ~12us) needs one 5-slot weight slice and consumes x rows progressively,
            # so row-chunked first tiles + the (g0,oc4=0) weight slots gate startup.
            W1 = 5  # slots per (g, oc4) weight slice
            WG = OC4 * W1  # slots per group
            R1, R2, R3 = 10, 26, 42  # x-tile row chunk boundaries

            xt = {}
            t1_first = xp.tile([128, 2, PH, PW], F8, tag="t1_0_0")
            t2_first = xp.tile([128, 2, PH, PW], F8, tag="t2_0_0")
            xt[0, 0] = (t1_first, t2_first)

            # All startup-critical chunks ride the fast HWDGE (sync) queue,
            # t1/t2 interleaved per row-chunk; scl first so the ACT table load
            # completes before the first psum is ready. SWDGE (gpsimd) ramps
            # slowly, so it only carries weights needed >=12us in.
            nc.sync.dma_start(scl[:], scl_d[:])
            nc.sync.dma_start(wts[:, 0:W1], wts_d[:, 0:W1])
            nc.sync.dma_start(t1_first[:, :, :R1], xt1_d[0, 0, :, :, :R1])
            nc.sync.dma_start(t2_first[:, :, :R1], xt2_d[0, 0, :, :, :R1])
            nc.sync.dma_start(t1_first[:, :, R1:R2], xt1_d[0, 0, :, :, R1:R2])
            nc.sync.dma_start(t2_first[:, :, R1:R2], xt2_d[0, 0, :, :, R1:R2])
            nc.sync.dma_start(t1_first[:, :, R2:R3], xt1_d[0, 0, :, :, R2:R3])
            nc.sync.dma_start(t2_first[:, :, R2:R3], xt2_d[0, 0, :, :, R2:R3])
            nc.sync.dma_start(t1_first[:, :, R3:], xt1_d[0, 0, :, :, R3:])
            nc.sync.dma_start(t2_first[:, :, R3:], xt2_d[0, 0, :, :, R3:])

            for i in range(1, OC4):
                nc.gpsimd.dma_start(wts[:, i * W1 : (i + 1) * W1], wts_d[:, i * W1 : (i + 1) * W1])
            nc.gpsimd.dma_start(wts[:, WG : 2 * WG], wts_d[:, WG : 2 * WG])

            for img in range(BPC):
                for g in range(G):
                    if (img, g) in xt:
                        continue
                    t1 = xp.tile([128, 2, PH, PW], F8, tag=f"t1_{img}_{g}")
                    t2 = xp.tile([128, 2, PH, PW], F8, tag=f"t2_{img}_{g}")
                    nc.sync.dma_start(t1[:], xt1_d[img, g])
                    nc.gpsimd.dma_start(t2[:], xt2_d[img, g])
                    xt[img, g] = (t1, t2)
                    if (img, g) == (0, 1):
                        # remaining weights after the (0,1) x tiles
                        nc.gpsimd.dma_start(wts[:, 2 * WG :], wts_d[:, 2 * WG :])

            def wslice(g, oc4, s):
                i = ((g * OC4) + oc4) * 5 + s
                return wts[:, i]

            for img in range(BPC):
                # one contiguous accumulator per och-half: [128, SP*448]
                acc = {}
                for oct in range(2):
                    acc_t = accp.tile([128, SP * NCOL], F16, tag=f"acc{oct}")
                    acc[oct] = acc_t

                for g in range(G):
                    t1, t2 = xt[img, g]
                    for oc4 in range(OC4):
                        iscl = g * OC4 + oc4
                        ratio_ap = scl[:, iscl : iscl + 1]
                        c_ap = scl[:, G * OC4 + iscl : G * OC4 + iscl + 1]
                        for sp in range(SP):
                            r0 = sp * ROWS
                            ps = pp.tile([128, NCOL], F32, tag="ps")
                            for s in range(3):
                                nc.tensor.matmul(
                                    ps[:],
                                    wslice(g, oc4, s),
                                    t1[:, :, r0 + s : r0 + s + ROWS, 0:W],
                                    start=(s == 0),
                                    stop=False,
                                    perf_mode=DR,
                                )
                            nc.tensor.matmul(
                                ps[:],
                                wslice(g, oc4, 3),
                                t2[:, :, r0 : r0 + ROWS, 2:PW],
                                start=False,
                                stop=False,
                                perf_mode=DR,
                            )
                            nc.tensor.matmul(
                                ps[:],
                                wslice(g, oc4, 4),
                                t1[:, :, r0 + 2 : r0 + 2 + ROWS, 2:PW],
                                start=False,
                                stop=True,
                                perf_mode=DR,
                            )
                            q8 = qp.tile([128, NCOL], I8, tag="q8")
                            nc.scalar.activation(
                                q8[:],
                                ps[:],
                                mybir.ActivationFunctionType.Copy,
                                bias=0.0,
                                scale=ratio_ap,
                            )
                            a = acc[oc4 % 2][:, sp * NCOL : (sp + 1) * NCOL]
                            if g == 0 and oc4 < 2:
                                nc.vector.tensor_scalar(
                                    a, q8[:], c_ap, None, mybir.AluOpType.mult
                                )
                            else:
                                nc.vector.scalar_tensor_tensor(
                                    a,
                                    q8[:],
                                    c_ap,
                                    a,
                                    mybir.AluOpType.mult,
                                    mybir.AluOpType.add,
                                )

                # Batched output DMAs. oct0 completes ~5us before oct1 (oc4
                # sweep order), so it ships whole; the final image's oct1 ships
                # in sp-pairs as they complete so the post-compute tail is only
                # the last 116KB piece.
                nc.sync.dma_start(out_d[img, 0], acc[0][:])
                if img < BPC - 1:
                    nc.gpsimd.dma_start(out_d[img, 1], acc[1][:])
                else:
                    for lo, hi, eng in ((0, 2, nc.gpsimd), (2, 4, nc.sync), (4, 6, nc.gpsimd), (6, 7, nc.sync)):
                        eng.dma_start(
                            out_d[img, 1, :, lo:hi],
                            acc[1][:, lo * NCOL : hi * NCOL],
                        )

    nc.compile()
    return nc


def _prepare(x, weight, w_scale, ps_scale_p, ps_scale_n):
    x = np.asarray(x, np.float32)
    weight = np.asarray(weight, np.float32)
    w_scale = np.asarray(w_scale, np.float32)
    ps_scale_p = np.asarray(ps_scale_p, np.float32)
    ps_scale_n = np.asarray(ps_scale_n, np.float32)

    # --- weight levels (exact f32 math matching the reference LSQ) ---
    wg = weight.reshape(OC, G, CG, K, K).transpose(1, 0, 2, 3, 4)  # [G,O,cg,k,k]
    s_w = w_scale.reshape(G, 1, 1, 1, 1)
    lvl_p = np.round(np.clip(np.maximum(wg, 0) / s_w, 0.0, float(QP_W))).astype(np.float32)
    lvl_n = np.round(np.clip(np.maximum(-wg, 0) / s_w, 0.0, float(QP_W))).astype(np.float32)
    LV = np.concatenate([lvl_p, lvl_n], axis=1)  # [G, 512, cg, 3, 3]

    # lhsT tiles [K=128, M=128] per (g, oc4, slot); both DoubleRow lanes get the
    # same integer-level weights (lane 0 contracts x_hi, lane 1 x_lo).
    wts = np.zeros((G, OC4, 5, 128, 128), np.float32)
    for g in range(G):
        for oc4 in range(OC4):
            t = LV[g, oc4 * 128 : (oc4 + 1) * 128]  # [128 och, cg, 3, 3]
            for s in range(3):  # taps (s,0)+(s,1)
                wts[g, oc4, s, :CG] = t[:, :, s, 0].T
                wts[g, oc4, s, CG:] = t[:, :, s, 1].T
            wts[g, oc4, 3, :CG] = t[:, :, 0, 2].T  # taps (0,2)+(1,2) via T2
            wts[g, oc4, 3, CG:] = t[:, :, 1, 2].T
            wts[g, oc4, 4, :CG] = t[:, :, 2, 2].T  # tap (2,2), upper half zero
    # -> [128 K, NSL, 2 lanes, 128 M]
    wflat = wts.transpose(3, 0, 1, 2, 4).reshape(128, NSL, 1, 128)
    wts_flat = np.ascontiguousarray(
        np.broadcast_to(wflat, (128, NSL, 2, 128))
    ).astype(NP_F8)

    # --- scales: ratio = s_w/s_ps ; c = +-s_ps ---
    scl = np.zeros((128, 2 * G * OC4), np.float32)
    for g in range(G):
        for oc4 in range(OC4):
            s_ps = ps_scale_p[g] if oc4 < 2 else ps_scale_n[g]
            sign = 1.0 if oc4 < 2 else -1.0
            scl[:, g * OC4 + oc4] = np.float32(w_scale[g]) / np.float32(s_ps)
            scl[:, G * OC4 + g * OC4 + oc4] = np.float32(sign) * np.float32(s_ps)

    # --- exact two-lane fp8 split of x: residual encoding, lanes e4m3-exact ---
    a = x.astype(NP_F8).astype(np.float32)
    b = (x - a).astype(NP_F8).astype(np.float32)
    # padded, shifted lanes: [B, G, 128 part, 2 lane, 58, 58]
    # K-halves: [A | A+1col] for T1, [A | A+1row] for T2 (flat shifts by 1 / by PW;
    # the one flat-shift row-crossing read in pass 4's upper half has zero weights).
    T1 = np.zeros((B, G, 128, 2, FLAT), NP_F8)
    T2 = np.zeros((B, G, 128, 2, FLAT), NP_F8)
    for lane, xl in ((0, a), (1, b)):
        xp8 = np.zeros((B, IC, PH, PW), NP_F8)
        xp8[:, :, 1 : H + 1, 1 : W + 1] = xl.astype(NP_F8)
        Af = xp8.reshape(B, G, CG, FLAT)
        T1[:, :, :CG, lane] = Af
        T1[:, :, CG:, lane, : FLAT - 1] = Af[..., 1:]
        T2[:, :, :CG, lane] = Af
        T2[:, :, CG:, lane, : FLAT - PW] = Af[..., PW:]
    T1 = T1.reshape(B, G, 128, 2, PH, PW)
    T2 = T2.reshape(B, G, 128, 2, PH, PW)

    return T1, T2, wts_flat, scl


def kernel(x, weight, w_scale, ps_scale_p, ps_scale_n, _trace=False, _tmpdir=None):
    T1, T2, wts_flat, scl = _prepare(x, weight, w_scale, ps_scale_p, ps_scale_n)

    if "nc" not in _nc_cache:
        _nc_cache["nc"] = _build_nc()
    nc = _nc_cache["nc"]

    in_maps = []
    for c in range(N_CORES):
        sl = slice(c * BPC, (c + 1) * BPC)
        in_maps.append(
            {
                "xt1": np.ascontiguousarray(T1[sl]),
                "xt2": np.ascontiguousarray(T2[sl]),
                "wts": wts_flat,
                "scl": scl,
            }
        )

    kwargs = {}
    if _trace:
        kwargs.update(trace=True, tmpdir=_tmpdir, trace_cores=[0])
    res = run_bass_kernel_spmd(nc, in_maps, core_ids=list(range(N_CORES)), **kwargs)

    out = np.concatenate([r["out"] for r in res.results], axis=0)  # [16, 2, 128, 7, 448] fp16
    final = out.reshape(B, OC, H, W).astype(np.float32)
    if _trace:
        kernel._last_results = res
    return final


# revision 12
# speedup vs baseline: 1.0533x; 1.0533x over previous
"""Trainium2 Bass kernel for nn_Conv4Pim_group_arr_v3 (PIM-style grouped quantized conv).

Computation (see reference):
  - x [16,256,56,56] f32, weight [256,256,3,3], per-group (G=4, 64 ic each) LSQ
    quantization: weights to integer levels {0..3} (pos/neg split), partial-sum conv
    outputs rounded to int levels in [-128,127] and rescaled, accumulated over groups.

Strategy: data-parallel over batch (2 images per core, 8 cores, no collectives).
fp8 DoubleRow matmuls: x is split exactly into two e4m3 lanes (x ~ a/4 + b/128 with
a,b integers, both lanes exact in fp8; |x|>4.125 tail uses an even grid, still exact),
weights are integer levels {0..3} (exact in fp8). Each DoubleRow pass contracts both
lanes (2 K-tiles of 128) per column, so full x precision costs no extra passes.

x is stored as a padded [58 rows, 58 cols] grid per (img, group), two shifted copies
(T1 = [A | A+1col] and T2 = [A | A+1row] stacked in the K partition dim) so conv taps
pair up. All matmul reads are rectangular [8 rows, 56 cols] slices -> 448 columns per
pass (no pad-column waste).

Per core, per (img, group, och-tile-of-512, sptile-of-8-rows):
  - 5 fp8 DoubleRow matmuls accumulate the 3x3 conv into one PSUM tile [128, 448].
  - ACT: Copy(psum * (w_scale/ps_scale)) with int8 output = round-half-even +
    saturate to [-128,127] in one op == the LSQ psum quantizer.
  - DVE scalar_tensor_tensor: acc_fp16 += q_int8 * (+-ps_scale).
Output fp16 -> host f32.
"""

import numpy as np
import ml_dtypes

import concourse.mybir as mybir
import concourse.tile as tile
from concourse import bacc
from concourse.bass_utils import run_bass_kernel_spmd

F32 = mybir.dt.float32
F16 = mybir.dt.float16
F8 = mybir.dt.float8e4
I8 = mybir.dt.int8
NP_F8 = ml_dtypes.float8_e4m3

B, IC, H, W = 16, 256, 56, 56
OC = 256
G = 4
CG = 64  # ic per group
K = 3
QP_W = 3  # 2**2 - 1
N_CORES = 8
BPC = B // N_CORES  # images per core

PW = W + 2  # 58 padded width
PH = H + 2  # 58 padded height
FLAT = PW * PH  # 3364
SP = 7  # spatial tiles of 8 output rows
ROWS = 8
NCOL = ROWS * W  # 448 columns per psum tile
OC4 = 4  # och tiles of 128 over 512 (pos|neg x 256)
NSL = G * OC4 * 5  # weight slices, each [128 K, 2 lanes, 128 M]

_nc_cache = {}


def _build_nc():
    nc = bacc.Bacc(
        "TRN2",
        target_bir_lowering=False,
        debug=False,
        enable_asserts=True,
        num_devices=N_CORES,
    )

    xt1_d = nc.dram_tensor("xt1", [BPC, G, 128, 2, PH, PW], F8, kind="ExternalInput").ap()
    xt2_d = nc.dram_tensor("xt2", [BPC, G, 128, 2, PH, PW], F8, kind="ExternalInput").ap()
    wts_d = nc.dram_tensor("wts", [128, NSL, 2, 128], F8, kind="ExternalInput").ap()
    scl_d = nc.dram_tensor("scl", [128, 2 * G * OC4], F32, kind="ExternalInput").ap()
    out_d = nc.dram_tensor("out", [BPC, 2, 128, SP, NCOL], F16, kind="ExternalOutput").ap()

    DR = mybir.MatmulPerfMode.DoubleRow

    with tile.TileContext(nc) as tc:
        with (
            tc.tile_pool(name="xp", bufs=1) as xp,
            tc.tile_pool(name="wp", bufs=1) as wp,
            tc.tile_pool(name="accp", bufs=2) as accp,
            tc.tile_pool(name="qp", bufs=8) as qp,
            tc.tile_pool(name="psum", bufs=8, space="PSUM") as pp,
        ):
            wts = wp.tile([128, NSL, 2, 128], F8, tag="wts")
            scl = wp.tile([128, 2 * G * OC4], F32, tag="scl")
            # Startup-critical DMA schedule over two queues (sync = HWDGE, gpsimd =
            # SWDGE). The (img0, g0) block runs oc4-outer: each oc4 sweep (7 sp tiles,
            # ~12us) needs one 5-slot weight slice and consumes x rows progressively,
            # so row-chunked first tiles + the (g0,oc4=0) weight slots gate startup.
            W1 = 5  # slots per (g, oc4) weight slice
            WG = OC4 * W1  # slots per group
            R1, R2, R3 = 10, 26, 42  # x-tile row chunk boundaries

            xt = {}
            t1_first = xp.tile([128, 2, PH, PW], F8, tag="t1_0_0")
            t2_first = xp.tile([128, 2, PH, PW], F8, tag="t2_0_0")
            xt[0, 0] = (t1_first, t2_first)

            # Startup-critical path on the fast HWDGE (sync) queue: scl first
            # (ACT table load), the (g0,oc4=0) weight slice, then t1 chunks;
            # t2's first chunk also rides sync to dodge the SWDGE spin-up
            # (~11.5us), the rest of t2 streams on gpsimd in parallel.
            nc.sync.dma_start(scl[:], scl_d[:])
            nc.sync.dma_start(wts[:, 0:W1], wts_d[:, 0:W1])
            nc.sync.dma_start(t1_first[:, :, :R1], xt1_d[0, 0, :, :, :R1])
            nc.sync.dma_start(t2_first[:, :, :R1], xt2_d[0, 0, :, :, :R1])
            nc.sync.dma_start(t1_first[:, :, R1:R2], xt1_d[0, 0, :, :, R1:R2])
            nc.sync.dma_start(t1_first[:, :, R2:R3], xt1_d[0, 0, :, :, R2:R3])
            nc.sync.dma_start(t1_first[:, :, R3:], xt1_d[0, 0, :, :, R3:])

            nc.gpsimd.dma_start(t2_first[:, :, R1:R2], xt2_d[0, 0, :, :, R1:R2])
            nc.gpsimd.dma_start(t2_first[:, :, R2:R3], xt2_d[0, 0, :, :, R2:R3])
            nc.gpsimd.dma_start(t2_first[:, :, R3:], xt2_d[0, 0, :, :, R3:])
            for i in range(1, OC4):
                nc.gpsimd.dma_start(wts[:, i * W1 : (i + 1) * W1], wts_d[:, i * W1 : (i + 1) * W1])
            nc.gpsimd.dma_start(wts[:, WG : 2 * WG], wts_d[:, WG : 2 * WG])

            for img in range(BPC):
                for g in range(G):
                    if (img, g) in xt:
                        continue
                    t1 = xp.tile([128, 2, PH, PW], F8, tag=f"t1_{img}_{g}")
                    t2 = xp.tile([128, 2, PH, PW], F8, tag=f"t2_{img}_{g}")
                    nc.sync.dma_start(t1[:], xt1_d[img, g])
                    nc.gpsimd.dma_start(t2[:], xt2_d[img, g])
                    xt[img, g] = (t1, t2)
                    if (img, g) == (0, 1):
                        # remaining weights after the (0,1) x tiles
                        nc.gpsimd.dma_start(wts[:, 2 * WG :], wts_d[:, 2 * WG :])

            def wslice(g, oc4, s):
                i = ((g * OC4) + oc4) * 5 + s
                return wts[:, i]

            for img in range(BPC):
                # one contiguous accumulator per och-half: [128, SP*448]
                acc = {}
                for oct in range(2):
                    acc_t = accp.tile([128, SP * NCOL], F16, tag=f"acc{oct}")
                    acc[oct] = acc_t

                for g in range(G):
                    t1, t2 = xt[img, g]
                    for oc4 in range(OC4):
                        iscl = g * OC4 + oc4
                        ratio_ap = scl[:, iscl : iscl + 1]
                        c_ap = scl[:, G * OC4 + iscl : G * OC4 + iscl + 1]
                        for sp in range(SP):
                            r0 = sp * ROWS
                            ps = pp.tile([128, NCOL], F32, tag="ps")
                            for s in range(3):
                                nc.tensor.matmul(
                                    ps[:],
                                    wslice(g, oc4, s),
                                    t1[:, :, r0 + s : r0 + s + ROWS, 0:W],
                                    start=(s == 0),
                                    stop=False,
                                    perf_mode=DR,
                                )
                            nc.tensor.matmul(
                                ps[:],
                                wslice(g, oc4, 3),
                                t2[:, :, r0 : r0 + ROWS, 2:PW],
                                start=False,
                                stop=False,
                                perf_mode=DR,
                            )
                            nc.tensor.matmul(
                                ps[:],
                                wslice(g, oc4, 4),
                                t1[:, :, r0 + 2 : r0 + 2 + ROWS, 2:PW],
                                start=False,
                                stop=True,
                                perf_mode=DR,
                            )
                            q8 = qp.tile([128, NCOL], I8, tag="q8")
                            nc.scalar.activation(
                                q8[:],
                                ps[:],
                                mybir.ActivationFunctionType.Copy,
                                bias=0.0,
                                scale=ratio_ap,
                            )
                            a = acc[oc4 % 2][:, sp * NCOL : (sp + 1) * NCOL]
                            if g == 0 and oc4 < 2:
                                nc.vector.tensor_scalar(
                                    a, q8[:], c_ap, None, mybir.AluOpType.mult
                                )
                            else:
                                nc.vector.scalar_tensor_tensor(
                                    a,
                                    q8[:],
                                    c_ap,
                                    a,
                                    mybir.AluOpType.mult,
                                    mybir.AluOpType.add,
                                )

                # Batched output DMAs. oct0 completes ~5us before oct1 (oc4
                # sweep order), so it ships whole; the final image's oct1 ships
                # in sp-pairs as they complete so the post-compute tail is only
                # the last 116KB piece.
                nc.sync.dma_start(out_d[img, 0], acc[0][:])
                if img < BPC - 1:
                    nc.gpsimd.dma_start(out_d[img, 1], acc[1][:])
                else:
                    for lo, hi, eng in ((0, 2, nc.gpsimd), (2, 4, nc.sync), (4, 6, nc.gpsimd), (6, 7, nc.sync)):
                        eng.dma_start(
                            out_d[img, 1, :, lo:hi],
                            acc[1][:, lo * NCOL : hi * NCOL],
                        )

    nc.compile()
    return nc


def _prepare(x, weight, w_scale, ps_scale_p, ps_scale_n):
    x = np.asarray(x, np.float32)
    weight = np.asarray(weight, np.float32)
    w_scale = np.asarray(w_scale, np.float32)
    ps_scale_p = np.asarray(ps_scale_p, np.float32)
    ps_scale_n = np.asarray(ps_scale_n, np.float32)

    # --- weight levels (exact f32 math matching the reference LSQ) ---
    wg = weight.reshape(OC, G, CG, K, K).transpose(1, 0, 2, 3, 4)  # [G,O,cg,k,k]
    s_w = w_scale.reshape(G, 1, 1, 1, 1)
    lvl_p = np.round(np.clip(np.maximum(wg, 0) / s_w, 0.0, float(QP_W))).astype(np.float32)
    lvl_n = np.round(np.clip(np.maximum(-wg, 0) / s_w, 0.0, float(QP_W))).astype(np.float32)
    LV = np.concatenate([lvl_p, lvl_n], axis=1)  # [G, 512, cg, 3, 3]

    # lhsT tiles [K=128, M=128] per (g, oc4, slot); both DoubleRow lanes get the
    # same integer-level weights (lane 0 contracts x_hi, lane 1 x_lo).
    wts = np.zeros((G, OC4, 5, 128, 128), np.float32)
    for g in range(G):
        for oc4 in range(OC4):
            t = LV[g, oc4 * 128 : (oc4 + 1) * 128]  # [128 och, cg, 3, 3]
            for s in range(3):  # taps (s,0)+(s,1)
                wts[g, oc4, s, :CG] = t[:, :, s, 0].T
                wts[g, oc4, s, CG:] = t[:, :, s, 1].T
            wts[g, oc4, 3, :CG] = t[:, :, 0, 2].T  # taps (0,2)+(1,2) via T2
            wts[g, oc4, 3, CG:] = t[:, :, 1, 2].T
            wts[g, oc4, 4, :CG] = t[:, :, 2, 2].T  # tap (2,2), upper half zero
    # -> [128 K, NSL, 2 lanes, 128 M]
    wflat = wts.transpose(3, 0, 1, 2, 4).reshape(128, NSL, 1, 128)
    wts_flat = np.ascontiguousarray(
        np.broadcast_to(wflat, (128, NSL, 2, 128))
    ).astype(NP_F8)

    # --- scales: ratio = s_w/s_ps ; c = +-s_ps ---
    scl = np.zeros((128, 2 * G * OC4), np.float32)
    for g in range(G):
        for oc4 in range(OC4):
            s_ps = ps_scale_p[g] if oc4 < 2 else ps_scale_n[g]
            sign = 1.0 if oc4 < 2 else -1.0
            scl[:, g * OC4 + oc4] = np.float32(w_scale[g]) / np.float32(s_ps)
            scl[:, G * OC4 + g * OC4 + oc4] = np.float32(sign) * np.float32(s_ps)

    # --- exact two-lane fp8 split of x: residual encoding, lanes e4m3-exact ---
    a = x.astype(NP_F8).astype(np.float32)
    b = (x - a).astype(NP_F8).astype(np.float32)
    # padded, shifted lanes: [B, G, 128 part, 2 lane, 58, 58]
    # K-halves: [A | A+1col] for T1, [A | A+1row] for T2 (flat shifts by 1 / by PW;
    # the one flat-shift row-crossing read in pass 4's upper half has zero weights).
    T1 = np.zeros((B, G, 128, 2, FLAT), NP_F8)
    T2 = np.zeros((B, G, 128, 2, FLAT), NP_F8)
    for lane, xl in ((0, a), (1, b)):
        xp8 = np.zeros((B, IC, PH, PW), NP_F8)
        xp8[:, :, 1 : H + 1, 1 : W + 1] = xl.astype(NP_F8)
        Af = xp8.reshape(B, G, CG, FLAT)
        T1[:, :, :CG, lane] = Af
        T1[:, :, CG:, lane, : FLAT - 1] = Af[..., 1:]
        T2[:, :, :CG, lane] = Af
        T2[:, :, CG:, lane, : FLAT - PW] = Af[..., PW:]
    T1 = T1.reshape(B, G, 128, 2, PH, PW)
    T2 = T2.reshape(B, G, 128, 2, PH, PW)

    return T1, T2, wts_flat, scl


def kernel(x, weight, w_scale, ps_scale_p, ps_scale_n, _trace=False, _tmpdir=None):
    T1, T2, wts_flat, scl = _prepare(x, weight, w_scale, ps_scale_p, ps_scale_n)

    if "nc" not in _nc_cache:
        _nc_cache["nc"] = _build_nc()
    nc = _nc_cache["nc"]

    in_maps = []
    for c in range(N_CORES):
        sl = slice(c * BPC, (c + 1) * BPC)
        in_maps.append(
            {
                "xt1": np.ascontiguousarray(T1[sl]),
                "xt2": np.ascontiguousarray(T2[sl]),
                "wts": wts_flat,
                "scl": scl,
            }
        )

    kwargs = {}
    if _trace:
        kwargs.update(trace=True, tmpdir=_tmpdir, trace_cores=[0])
    res = run_bass_kernel_spmd(nc, in_maps, core_ids=list(range(N_CORES)), **kwargs)

    out = np.concatenate([r["out"] for r in res.results], axis=0)  # [16, 2, 128, 7, 448] fp16
    final = out.reshape(B, OC, H, W).astype(np.float32)
    if _trace:
        kernel._last_results = res
    return final
